# revision 14
# baseline (speedup 1.0000x reference)
"""Multi-head attention (QKV projection + softmax attention) on 8 TRN2 NeuronCores.

Problem: x[2,2048,1024] @ W_qkv[1024,3072] + b -> q,k,v (16 heads, d=64);
out = softmax(q k^T / sqrt(d)) v, returned as [2,2048,1024] fp32.

Sharding: head-parallel. Core c owns heads {2c, 2c+1} for both batches.
Each core computes the QKV projection only for its own heads' columns of
W_qkv and runs its 4 (batch, head) attention instances locally. No
collectives; host gathers/normalizes/concatenates.

Device-side layout choices (per core):
  - x is fed pre-transposed (xT [B, C, N]) so the projection can produce
    qT/kT [d, n] (head dim on partitions, h0 on partitions 0-63, h1 on
    64-127) directly with full-width matmuls.
  - Attention computes S^T = K Q^T per head via matmul(lhsT=kT, rhs=qT)
    (contraction over d=64; the two heads run row-tiled concurrently in the
    PE array), softmax numerator via one ACT exp per j-chunk covering both
    heads, and out^T = V^T E^T via matmul(lhsT=[v|1], rhs=E^T) accumulated
    over j in PSUM. The appended ones-column yields the softmax denominator
    as row 64 of the accumulator; normalization happens on host.
  - All matmuls use float32r (full fp32 data, reduced-precision multiply)
    which streams at bf16 rate for moving dims >= 256.
"""

import math
import os
from contextlib import ExitStack
from dataclasses import dataclass

import numpy as np

import concourse.bass as bass
import concourse.tile as tile
from concourse import bacc, mybir
from concourse.bass_utils import run_bass_kernel_spmd
from concourse.masks import make_identity

F32 = mybir.dt.float32
F32R = mybir.dt.float32r


@dataclass(frozen=True)
class Cfg:
    B: int = 2          # batches
    N: int = 2048       # sequence length
    C: int = 1024       # model dim (contraction dim of the projection)
    D: int = 64         # head dim
    IT: int = 512       # i-tile (query block, moving-dim of S^T / PV matmuls)
    P: int = 128        # partitions

    @property
    def KO(self):       # k-chunks in the projection contraction
        return self.C // self.P

    @property
    def NJ(self):       # key chunks of 128
        return self.N // self.P

    @property
    def NI(self):       # query tiles
        return self.N // self.IT

    @property
    def HD2(self):      # two heads stacked on partitions
        return 2 * self.D


# Walrus requires every producer of an fp32r-matmul operand to emit
# fp32r-rounded data. DMA loads don't round, so DMA-fed matmul operands are
# staged into an fp32 scratch tile and cast into their f32r tile on the
# vector engine (the cast IS the rounding).


def build_attention(tc: tile.TileContext, io: dict, cfg: Cfg):
    nc = tc.nc
    P, D, IT = cfg.P, cfg.D, cfg.IT
    xT, wq, wk, wv, bq, bk, bv, outT = (
        io["xT"], io["wq"], io["wk"], io["wv"],
        io["bq"], io["bk"], io["bv"], io["outT"],
    )

    with ExitStack() as ctx:
        consts = ctx.enter_context(tc.tile_pool(name="consts", bufs=1))
        xpool = ctx.enter_context(tc.tile_pool(name="xpool", bufs=1))
        xstage = ctx.enter_context(tc.tile_pool(name="xstage", bufs=2))
        qkv = ctx.enter_context(tc.tile_pool(name="qkv", bufs=2))
        epool = ctx.enter_context(tc.tile_pool(name="epool", bufs=3))
        opool = ctx.enter_context(tc.tile_pool(name="opool", bufs=4))
        ppool = ctx.enter_context(tc.tile_pool(name="ppool", bufs=2, space="PSUM"))
        spool = ctx.enter_context(tc.tile_pool(name="spool", bufs=2, space="PSUM"))
        apool = ctx.enter_context(tc.tile_pool(name="apool", bufs=2, space="PSUM"))

        identity = consts.tile([P, P], F32)
        make_identity(nc, identity)

        # weights [ki, ko, m] and biases [p, 1]
        w_sb = {}
        for name, wdram in (("q", wq), ("k", wk), ("v", wv)):
            ws = xstage.tile([P, max(cfg.N, cfg.KO * cfg.HD2)], F32, tag="stage",
                             name=f"ws_{name}")
            ws = ws[:, :cfg.KO * cfg.HD2].rearrange(
                "p (ko m) -> p ko m", ko=cfg.KO
            )
            nc.sync.dma_start(
                out=ws,
                in_=wdram.rearrange("(ko ki) m -> ki ko m", ki=P),
            )
            w_sb[name] = consts.tile([P, cfg.KO, cfg.HD2], F32R, name=f"w_{name}")
            nc.vector.tensor_copy(out=w_sb[name][:], in_=ws)
        b_sb = {}
        for name, bdram in (("q", bq), ("k", bk), ("v", bv)):
            b_sb[name] = consts.tile([cfg.HD2, 1], F32, name=f"b_{name}")
            nc.sync.dma_start(out=b_sb[name], in_=bdram)

        for b in range(cfg.B):
            # ---- load x^T for this batch: [ci, ko, n] ----
            x_sb = xpool.tile([P, cfg.KO, cfg.N], F32R, tag="xT")
            for ko in range(cfg.KO):
                xs = xstage.tile([P, max(cfg.N, cfg.KO * cfg.HD2)], F32,
                                 tag="stage", name=f"xs_{b}_{ko}")
                xs = xs[:, :cfg.N]
                nc.sync.dma_start(
                    out=xs,
                    in_=xT[b, ko * P:(ko + 1) * P, :],
                )
                nc.vector.tensor_copy(out=x_sb[:, ko], in_=xs)

            # ---- projections: qT/kT [2*D partitions, N] f32r, vT fp32 ----
            proj = {}
            for name in ("q", "k", "v"):
                dt = F32 if name == "v" else F32R
                dstT = qkv.tile([cfg.HD2, cfg.N], dt, tag=f"{name}T", name=f"{name}T")
                proj[name] = dstT
                for it in range(cfg.N // IT):
                    ps = ppool.tile([P, IT], F32, tag="proj")
                    for ko in range(cfg.KO):
                        nc.tensor.matmul(
                            ps[:cfg.HD2],
                            lhsT=w_sb[name][:, ko],
                            rhs=x_sb[:, ko, it * IT:(it + 1) * IT],
                            start=(ko == 0),
                            stop=(ko == cfg.KO - 1),
                        )
                    nc.vector.tensor_scalar_add(
                        out=dstT[:, it * IT:(it + 1) * IT],
                        in0=ps[:cfg.HD2],
                        scalar1=b_sb[name],
                    )

            # ---- v_aug [j, (v_h0|1|v_h1|1)] via PE transpose of vT ----
            DA = D + 1  # head-dim columns + ones column
            v_aug = qkv.tile([P, cfg.NJ, 2 * DA], F32R, tag="vaug")
            ones_col = consts.tile([P, cfg.NJ], F32, name=f"ones_{b}", tag="ones")
            nc.vector.memset(ones_col, 1.0)
            nc.vector.tensor_copy(out=v_aug[:, :, D], in_=ones_col)
            nc.vector.tensor_copy(out=v_aug[:, :, DA + D], in_=ones_col)
            for jc in range(cfg.NJ):
                tp = ppool.tile([P, IT], F32, tag="proj")
                nc.tensor.transpose(
                    tp[:, :P], proj["v"][:, jc * P:(jc + 1) * P], identity
                )
                nc.vector.tensor_copy(
                    out=v_aug[:, jc, 0:D], in_=tp[:, 0:D]
                )
                nc.vector.tensor_copy(
                    out=v_aug[:, jc, DA:DA + D], in_=tp[:, D:cfg.HD2]
                )

            qT, kT = proj["q"], proj["k"]

            # ---- attention ----
            o_sb = [opool.tile([DA, cfg.N], F32, tag="o", name=f"o{h}") for h in range(2)]
            for it in range(cfg.NI):
                isl = slice(it * IT, (it + 1) * IT)
                acc = [apool.tile([DA, IT], F32, tag="acc", name=f"acc{h}") for h in range(2)]
                for j in range(cfg.NJ):
                    jsl = slice(j * P, (j + 1) * P)
                    s = spool.tile([P, 2 * IT], F32, tag="s")
                    for h in range(2):
                        hsl = slice(h * D, (h + 1) * D)
                        nc.tensor.matmul(
                            s[:, h * IT:(h + 1) * IT],
                            lhsT=kT[hsl, jsl],
                            rhs=qT[hsl, isl],
                        )
                    e = epool.tile([P, 2 * IT], F32R, tag="e")
                    nc.scalar.activation(
                        e, s, mybir.ActivationFunctionType.Exp
                    )
                    for h in range(2):
                        nc.tensor.matmul(
                            acc[h],
                            lhsT=v_aug[:, j, h * DA:(h + 1) * DA],
                            rhs=e[:, h * IT:(h + 1) * IT],
                            start=(j == 0),
                            stop=(j == cfg.NJ - 1),
                        )
                for h in range(2):
                    nc.vector.tensor_copy(out=o_sb[h][:, isl], in_=acc[h])
            for h in range(2):
                nc.sync.dma_start(out=outT[b, h], in_=o_sb[h])


def build_program(cfg: Cfg):
    nc = bacc.Bacc("TRN2", target_bir_lowering=False, debug=False)
    io = {
        "xT": nc.dram_tensor("xT", (cfg.B, cfg.C, cfg.N), F32, kind="ExternalInput").ap(),
        "wq": nc.dram_tensor("wq", (cfg.C, cfg.HD2), F32, kind="ExternalInput").ap(),
        "wk": nc.dram_tensor("wk", (cfg.C, cfg.HD2), F32, kind="ExternalInput").ap(),
        "wv": nc.dram_tensor("wv", (cfg.C, cfg.HD2), F32, kind="ExternalInput").ap(),
        "bq": nc.dram_tensor("bq", (cfg.HD2, 1), F32, kind="ExternalInput").ap(),
        "bk": nc.dram_tensor("bk", (cfg.HD2, 1), F32, kind="ExternalInput").ap(),
        "bv": nc.dram_tensor("bv", (cfg.HD2, 1), F32, kind="ExternalInput").ap(),
        "outT": nc.dram_tensor(
            "outT", (cfg.B, 2, cfg.D + 1, cfg.N), F32, kind="ExternalOutput"
        ).ap(),
    }
    with tile.TileContext(nc) as tc:
        build_attention(tc, io, cfg)
    nc.compile()
    return nc


def shard_inputs(x, W_qkv, b_qkv, n_cores=8):
    """Full inputs -> per-core in_maps (head-sharded, q pre-scaled)."""
    B, N, C = x.shape
    D = 64
    H = C // D
    heads_per_core = H // n_cores
    assert heads_per_core == 2
    scale = D ** -0.5
    xT = np.ascontiguousarray(np.transpose(x, (0, 2, 1)), dtype=np.float32)
    W = np.asarray(W_qkv, np.float32).reshape(C, 3, H, D)
    bias = np.asarray(b_qkv, np.float32).reshape(3, H, D)
    in_maps = []
    for c in range(n_cores):
        hs = slice(2 * c, 2 * c + 2)
        in_maps.append({
            "xT": xT,
            "wq": np.ascontiguousarray(W[:, 0, hs].reshape(C, 128) * scale),
            "wk": np.ascontiguousarray(W[:, 1, hs].reshape(C, 128)),
            "wv": np.ascontiguousarray(W[:, 2, hs].reshape(C, 128)),
            "bq": np.ascontiguousarray(bias[0, hs].reshape(128, 1) * scale),
            "bk": np.ascontiguousarray(bias[1, hs].reshape(128, 1)),
            "bv": np.ascontiguousarray(bias[2, hs].reshape(128, 1)),
        })
    return in_maps


def gather_output(results, B=2, N=2048, C=1024):
    """Per-core outT [B, 2, 65, N] -> full [B, N, C] (normalize + interleave)."""
    outs = []
    for res in results:
        oT = np.asarray(res["outT"], np.float32)   # [B, 2, 65, N]
        o = oT[:, :, :64, :] / oT[:, :, 64:65, :]
        outs.append(np.transpose(o, (0, 3, 1, 2)))  # [B, N, 2, 64]
    out = np.concatenate(outs, axis=2)              # [B, N, 16, 64]
    return np.ascontiguousarray(out.reshape(B, N, C))


_PROGRAM = None


def kernel(x, W_qkv, b_qkv):
    global _PROGRAM
    cfg = Cfg()
    x = np.asarray(x, np.float32)
    in_maps = shard_inputs(x, W_qkv, b_qkv)
    if _PROGRAM is None:
        _PROGRAM = build_program(cfg)
    res = run_bass_kernel_spmd(_PROGRAM, in_maps, core_ids=list(range(8)))
    return gather_output(res.results, cfg.B, cfg.N, cfg.C)


if __name__ == "__main__":
    rng = np.random.default_rng(0)
    x = rng.standard_normal((2, 2048, 1024), dtype=np.float32)
    W = rng.standard_normal((1024, 3072), dtype=np.float32) * (1024 ** -0.5)
    b = rng.standard_normal(3072, dtype=np.float32) * 0.01
    out = kernel(x, W, b)
    print(out.shape, out.dtype, float(np.abs(out).max()))
